# revision 23
# baseline (speedup 1.0000x reference)
"""Trainium2 Bass/Tile kernel: Longformer-4096 encoder + BiLSTM + event-pair head.

Sharding: sequence-parallel over 8 cores (512 tokens each) for the encoder with
per-layer halo AllGather; the BiLSTM runs chunk-parallel (C chunks per core per
direction with warmup ticks, all chunks batched into shared matmuls so the
recurrent weight stream amortizes); event pooling via host-built selection
matrices + AllReduce; pair MLP sharded 252 pairs/core.

Self-contained: hardcodes all shapes; builds per-core inputs on host (index
matrices, masks, weight layout transforms), runs one SPMD Bass program on 8
cores, gathers outputs on host.
"""

import sys
from contextlib import ExitStack
from dataclasses import dataclass

import numpy as np

for _p in ("/opt/trn_rl_repo", "/root/.axon_site/_ro/trn_rl_repo"):
    if _p not in sys.path:
        sys.path.append(_p)

import ml_dtypes  # noqa: E402
import concourse.bass as bass  # noqa: E402
import concourse.tile as tile  # noqa: E402
from concourse import bacc, mybir  # noqa: E402
from concourse import bass_utils  # noqa: E402
from concourse.bass import ds  # noqa: E402
from concourse.masks import make_identity  # noqa: E402

F32 = mybir.dt.float32
F32R = mybir.dt.float32r
BF16 = mybir.dt.bfloat16
I32 = mybir.dt.int32
AF = mybir.ActivationFunctionType
OP = mybir.AluOpType
BF = ml_dtypes.bfloat16


@dataclass(frozen=True)
class Cfg:
    S: int = 4096
    D: int = 768
    NH: int = 12
    DH: int = 64
    NL: int = 12
    FF: int = 3072
    WIN: int = 256          # one-sided attention window
    HID: int = 384
    NE: int = 64
    NPAIR: int = 2016
    VOCAB: int = 50265
    NC: int = 8             # cores
    C: int = 8              # LSTM chunks per core per direction
    WARM: int = 64          # LSTM warmup ticks
    PPC: int = 256          # padded pairs per core

    @property
    def R(self):            # tokens per core
        return self.S // self.NC

    @property
    def TB(self):           # 128-token blocks per core
        return self.R // 128

    @property
    def KB(self):           # 128-blocks of D
        return self.D // 128

    @property
    def FB(self):           # 128-blocks of FF
        return self.FF // 128

    @property
    def G4(self):
        return 4 * self.HID

    @property
    def GB(self):           # 128-blocks of 4*HID
        return self.G4 // 128

    @property
    def HB(self):           # 128-blocks of HID
        return self.HID // 128

    @property
    def T(self):            # LSTM chunk length
        return self.R // self.C

    @property
    def TICKS(self):
        return self.T + self.WARM

    @property
    def HHC(self):          # h-history cols (col 0 and col HHC-1 are zero-init)
        return self.TICKS + 2

    @property
    def EXT(self):          # lstm-stage extended tokens per core
        return self.R + 2 * self.WARM

    @property
    def RPC(self):          # real pairs per core
        return (self.NPAIR + self.NC - 1) // self.NC


# qc -> [(rb, mask_index)] ; mask tiles are per-core host data.
MASK_PLAN = {0: [(0, 0), (1, 1), (4, 2)],
             1: [(0, 3), (4, 4)],
             2: [(0, 5), (4, 6)],
             3: [(0, 7), (3, 8), (4, 9)]}
NMASK = 10


def _gate_perm(cfg):
    # pytorch gate order i,f,g,o -> our order i,f,o,g
    H = cfg.HID
    return np.concatenate([np.arange(0, H), np.arange(H, 2 * H),
                           np.arange(3 * H, 4 * H), np.arange(2 * H, 3 * H)])


# --------------------------------------------------------------------------
# Host-side input prep
# --------------------------------------------------------------------------

def _prep(inputs, cfg: Cfg):
    p = inputs["params"]
    L = p["layers"]
    f32 = lambda a: np.ascontiguousarray(np.asarray(a, np.float32))
    bf = lambda a: np.ascontiguousarray(np.asarray(a, np.float32).astype(BF))

    shared = {}
    shared["word_emb"] = f32(p["word_emb"])
    for nm in ("wq", "wk", "wv", "wo"):
        shared[nm] = bf(L[nm])                      # [NL, D, D]
    for nm in ("bq", "bk", "b1", "b2", "ln1_g", "ln1_b", "ln2_g", "ln2_b"):
        shared[nm] = f32(L[nm])
    # v-bias folded into the o-projection bias: (ctx+bv)@wo+bo = ctx@wo + (bo+bv@wo)
    bo_eff = np.asarray(L["bo"], np.float32) + np.einsum(
        "ld,lde->le", np.asarray(L["bv"], np.float32),
        np.asarray(L["wo"], np.float32))
    shared["bo"] = f32(bo_eff)
    shared["w1"] = bf(L["w1"])                      # [NL, D, FF]
    # 0.5 of tanh-gelu folded into w2
    shared["w2"] = bf(0.5 * np.asarray(L["w2"], np.float32))
    shared["ln_emb_g"] = f32(p["ln_emb_g"])
    shared["ln_emb_b"] = f32(p["ln_emb_b"])

    perm = _gate_perm(cfg)
    lp = p["lstm"]
    shared["wih_f"] = bf(np.asarray(lp["Wih_f"], np.float32)[perm].T)  # [D, G4]
    shared["wih_b"] = bf(np.asarray(lp["Wih_b"], np.float32)[perm].T)
    shared["whh_f"] = bf(np.asarray(lp["Whh_f"], np.float32)[perm].T)  # [HID, G4]
    shared["whh_b"] = bf(np.asarray(lp["Whh_b"], np.float32)[perm].T)
    shared["blstm_f"] = f32(np.asarray(lp["b_f"], np.float32)[perm])
    shared["blstm_b"] = f32(np.asarray(lp["b_b"], np.float32)[perm])

    shared["h1_w"] = bf(p["h1_w"])                  # [4D, D]
    shared["h1_b"] = f32(p["h1_b"])
    shared["h2_w"] = bf(p["h2_w"])                  # [D, 256]
    shared["h2_b"] = f32(p["h2_b"])
    shared["h3_w"] = bf(p["h3_w"])                  # [256, 2]
    shared["h3_b"] = f32(p["h3_b"])

    label_event = np.asarray(inputs["label_event"], np.int64)
    event_pairs = np.asarray(inputs["event_pairs"], np.int64)
    label_co = np.asarray(inputs["label_coreference"], np.int64)
    ids = np.asarray(inputs["input_ids"], np.int64)
    pos = f32(p["pos_emb"])

    starts, ends = label_event[:, 0], label_event[:, 1]
    shared["inv_len"] = (1.0 / (ends - starts).astype(np.float64)) \
        .astype(np.float32).reshape(cfg.NE, 1)

    S, R, NE = cfg.S, cfg.R, cfg.NE
    in_maps = []
    for c in range(cfg.NC):
        m = dict(shared)
        m["ids"] = ids[c * R:(c + 1) * R].astype(np.int32)
        m["pos_c"] = pos[c * R:(c + 1) * R]

        mk = np.zeros((NMASK, 128, 128), np.float32)
        for qc, lst in MASK_PLAN.items():
            for rb, mi in lst:
                qg = c * R + qc * 128 + np.arange(128)[None, :]
                kg = c * R - 256 + (qc + rb) * 128 + np.arange(128)[:, None]
                valid = (np.abs(kg - qg) <= cfg.WIN) & (kg >= 0) & (kg < S)
                mk[mi] = np.where(valid, 0.0, -1e9).astype(np.float32)
        m["masks"] = mk

        tglob = c * R + np.arange(R)[:, None]
        msel = (tglob >= starts[None, :]) & (tglob < ends[None, :])
        m["mt"] = msel.astype(np.float32).astype(BF)

        g1 = np.zeros((NE, cfg.PPC), np.float32)
        g2 = np.zeros((NE, cfg.PPC), np.float32)
        yw = np.zeros((2, cfg.PPC), np.float32)
        lo = c * cfg.RPC
        hi = min(lo + cfg.RPC, cfg.NPAIR)
        for j in range(lo, hi):
            g1[event_pairs[j, 0], j - lo] = 1.0
            g2[event_pairs[j, 1], j - lo] = 1.0
            yw[0, j - lo] = float(label_co[j])
            yw[1, j - lo] = 1.0
        m["g1t"] = g1.astype(BF)
        m["g2t"] = g2.astype(BF)
        m["yw"] = yw

        # LSTM warmup force-reset (edge cores): Xp := Xp*s + a on halo columns
        fxa = np.zeros((2, 128, cfg.GB), np.float32)
        fxs = np.ones((2, 128, 1), np.float32)
        if c == 0:
            fxs[0] = 0.0
            fxa[0, :, :9] = -50.0    # i,f,o blocks pinned very negative
        if c == cfg.NC - 1:
            fxs[1] = 0.0
            fxa[1, :, :9] = -50.0
        m["fixs"] = fxs
        m["fixa"] = fxa

        hv = np.ones((2, 128, 1), np.float32)
        if c == 0:
            hv[0] = 0.0
        if c == cfg.NC - 1:
            hv[1] = 0.0
        m["hok"] = hv

        m["pm1"] = np.array([[-1.0], [1.0]], np.float32)
        lsl = 2 * ((c - 1) % cfg.NC) + 1
        rsl = 2 * ((c + 1) % cfg.NC)
        hidx = np.zeros((128, 2, cfg.KB), np.int32)
        for kb in range(cfg.KB):
            hidx[:, 0, kb] = lsl * cfg.KB * 128 + kb * 128 + np.arange(128)
            hidx[:, 1, kb] = rsl * cfg.KB * 128 + kb * 128 + np.arange(128)
        m["hidx"] = hidx
        m["nbr"] = np.array([[2 * ((c - 1) % cfg.NC) + 1,
                              2 * ((c + 1) % cfg.NC)]], np.int32)
        in_maps.append(m)
    return in_maps


# --------------------------------------------------------------------------
# Device program
# --------------------------------------------------------------------------

def _ap(ap, offset_elems, dims, parts=None):
    """Raw AP view: keep (or override) partition dim, set free dims to
    [(step, count), ...], add element offset."""
    p0 = list(ap.ap[0]) if parts is None else list(parts)
    return bass.AP(tensor=ap.tensor, offset=ap.offset + offset_elems,
                   ap=[p0] + [list(x) for x in dims])


def build(cfg: Cfg):
    nc = bacc.Bacc("TRN2", target_bir_lowering=False, debug=False,
                   enable_asserts=False, num_devices=cfg.NC)
    D, R, KB, TB, NH, FB = cfg.D, cfg.R, cfg.KB, cfg.TB, cfg.NH, cfg.FB
    NL, FF, NE, G4, GB, HB = cfg.NL, cfg.FF, cfg.NE, cfg.G4, cfg.GB, cfg.HB
    C, T, WARM, TICKS, HHC, EXT = (cfg.C, cfg.T, cfg.WARM, cfg.TICKS, cfg.HHC,
                                   cfg.EXT)
    WIN, QC, PPC = cfg.WIN, R // 128, cfg.PPC

    t_in = lambda nm, shape, dt: nc.dram_tensor(nm, list(shape), dt,
                                                kind="ExternalInput").ap()
    word_emb = t_in("word_emb", (cfg.VOCAB, D), F32)
    wq = t_in("wq", (NL, D, D), BF16); wk = t_in("wk", (NL, D, D), BF16)
    wv = t_in("wv", (NL, D, D), BF16); wo = t_in("wo", (NL, D, D), BF16)
    w1 = t_in("w1", (NL, D, FF), BF16); w2 = t_in("w2", (NL, FF, D), BF16)
    bq = t_in("bq", (NL, D), F32); bk = t_in("bk", (NL, D), F32)
    bo = t_in("bo", (NL, D), F32)
    b1 = t_in("b1", (NL, FF), F32); b2 = t_in("b2", (NL, D), F32)
    ln1_g = t_in("ln1_g", (NL, D), F32); ln1_b = t_in("ln1_b", (NL, D), F32)
    ln2_g = t_in("ln2_g", (NL, D), F32); ln2_b = t_in("ln2_b", (NL, D), F32)
    lne_g = t_in("ln_emb_g", (D,), F32); lne_b = t_in("ln_emb_b", (D,), F32)
    wih = {0: t_in("wih_f", (D, G4), BF16), 1: t_in("wih_b", (D, G4), BF16)}
    whh = {0: t_in("whh_f", (cfg.HID, G4), BF16),
           1: t_in("whh_b", (cfg.HID, G4), BF16)}
    blstm = {0: t_in("blstm_f", (G4,), F32), 1: t_in("blstm_b", (G4,), F32)}
    h1w = t_in("h1_w", (4 * D, D), BF16); h1b = t_in("h1_b", (D,), F32)
    h2w = t_in("h2_w", (D, 256), BF16); h2b = t_in("h2_b", (256,), F32)
    h3w = t_in("h3_w", (256, 2), BF16); h3b = t_in("h3_b", (2,), F32)
    inv_len = t_in("inv_len", (NE, 1), F32)
    ids_d = t_in("ids", (R,), I32)
    pos_d = t_in("pos_c", (R, D), F32)
    masks_d = t_in("masks", (NMASK, 128, 128), F32)
    mt_d = t_in("mt", (R, NE), BF16)
    g1t_d = t_in("g1t", (NE, PPC), BF16)
    g2t_d = t_in("g2t", (NE, PPC), BF16)
    yw_d = t_in("yw", (2, PPC), F32)
    fixs_d = t_in("fixs", (2, 128, 1), F32)
    fixa_d = t_in("fixa", (2, 128, GB), F32)
    hok_d = t_in("hok", (2, 128, 1), F32)
    nbr_d = t_in("nbr", (1, 2), I32)
    pm1_d = t_in("pm1", (2, 1), F32)
    hidx_d = t_in("hidx", (128, 2, KB), I32)

    logits_o = nc.dram_tensor("logitsT", [2, PPC], F32,
                              kind="ExternalOutput").ap()
    loss_o = nc.dram_tensor("loss_part", [1, 1], F32, kind="ExternalOutput").ap()

    shsp = "Shared" if cfg.NC > 4 else "Local"
    cc_x_in = nc.dram_tensor("cc_x_in", [2, KB, 128, WIN], BF16,
                             kind="Internal").ap()
    cc_x_out = nc.dram_tensor("cc_x_out", [cfg.NC * 2, KB, 128, WIN], BF16,
                              kind="Internal", addr_space=shsp).ap()
    cc_h_in = nc.dram_tensor("cc_h_in", [2, KB, 128, WARM], BF16,
                             kind="Internal").ap()
    cc_h_out = nc.dram_tensor("cc_h_out", [cfg.NC * 2, KB, 128, WARM], BF16,
                              kind="Internal", addr_space=shsp).ap()
    cc_ev_in = nc.dram_tensor("cc_ev_in", [NE, D], F32, kind="Internal").ap()
    cc_ev_out = nc.dram_tensor("cc_ev_out", [NE, D], F32, kind="Internal",
                               addr_space=shsp).ap()
    groups = [list(range(cfg.NC))]

    with tile.TileContext(nc) as tc, ExitStack() as ctx:
        glob = ctx.enter_context(tc.tile_pool(name="glob", bufs=1))

        ident_f = glob.tile([128, 128], F32, tag="ident_f", name="ident_f")
        make_identity(nc, ident_f)
        ident_b = glob.tile([128, 128], BF16, tag="ident_b", name="ident_b")
        make_identity(nc, ident_b)
        ones_f = glob.tile([128, 1], F32, tag="ones_f", name="ones_f")
        nc.vector.memset(ones_f, 1.0)
        ones_r = glob.tile([1, 128], F32, tag="ones_r", name="ones_r")
        nc.vector.memset(ones_r, 1.0)
        ones_b = glob.tile([128, 1], BF16, tag="ones_b", name="ones_b")
        nc.vector.memset(ones_b, 1.0)
        ones_rb = glob.tile([1, 128], BF16, tag="ones_rb", name="ones_rb")
        nc.vector.memset(ones_rb, 1.0)
        hs_sum = glob.tile([128, KB, R], F32, tag="hs_sum", name="hs_sum")
        nc.vector.memset(hs_sum[:], 0.0)

        hidx_sb = glob.tile([128, 2, KB], I32, tag="hidx_sb", name="hidx_sb")
        nc.gpsimd.dma_start(out=hidx_sb[:], in_=hidx_d[:, :, :])
        ccx_rows = cc_x_out.rearrange("s k p w -> (s k p) w")
        cch_rows = cc_h_out.rearrange("s k p w -> (s k p) w")

        hok_sb = glob.tile([128, 2], F32, tag="hok_sb", name="hok_sb")
        nc.gpsimd.dma_start(out=hok_sb[:],
                            in_=hok_d.rearrange("s p one -> p (s one)"))

        # ============================ encoder ============================
        with tc.tile_pool(name="enc", bufs=1) as enc, \
             tc.tile_pool(name="wpan", bufs=8) as wpan, \
             tc.tile_pool(name="w1pan", bufs=6) as w1pan, \
             tc.tile_pool(name="biasp", bufs=2) as biasp, \
             tc.tile_pool(name="scr", bufs=3) as scr, \
             tc.tile_pool(name="psum", bufs=2, space="PSUM") as psum:

            h_res = enc.tile([128, KB, R], F32, tag="h_res", name="h_res")
            xbf = enc.tile([128, KB, R], BF16, tag="xbf", name="xbf")
            xh = {0: enc.tile([128, KB, WIN], BF16, tag="xh_l", name="xh_l"),
                  1: enc.tile([128, KB, WIN], BF16, tag="xh_r", name="xh_r")}
            kt_ext = enc.tile([128, KB, R + 2 * WIN], BF16, tag="kt_ext", name="kt_ext")
            v_ext = enc.tile([128, TB + 4, 12, 65], BF16, tag="v_ext", name="v_ext")
            qt = enc.tile([128, KB, R], BF16, tag="qt", name="qt")
            ctxT = enc.tile([128, KB, R], BF16, tag="ctxT", name="ctxT")
            gelu_sb = enc.tile([128, FB, R], BF16, tag="gelu_sb", name="gelu_sb")
            mask_sb = enc.tile([128, NMASK, 128], F32, tag="mask_sb", name="mask_sb")
            nc.gpsimd.dma_start(out=mask_sb[:],
                                in_=masks_d.rearrange("m p q -> p m q"))
            nc.vector.memset(v_ext[:, :, :, 64:65], 1.0)

            # ---------- embedding ----------
            ids_sb = enc.tile([128, TB], I32, tag="ids_sb", name="ids_sb")
            nc.gpsimd.dma_start(out=ids_sb[:],
                                in_=ids_d.rearrange("(b p) -> p b", p=128))
            for tb in range(TB):
                erow = scr.tile([128, D], F32, tag="erow", name="erow", bufs=2)
                nc.gpsimd.indirect_dma_start(
                    out=erow[:], out_offset=None, in_=word_emb[:, :],
                    in_offset=bass.IndirectOffsetOnAxis(
                        ap=ids_sb[:, tb:tb + 1], axis=0))
                prow = scr.tile([128, D], F32, tag="prow", name="prow", bufs=2)
                nc.sync.dma_start(out=prow[:],
                                  in_=pos_d[tb * 128:(tb + 1) * 128, :])
                nc.vector.tensor_add(erow[:], erow[:], prow[:])
                for kb in range(KB):
                    pt = psum.tile([128, 128], F32, tag="pproj", name="ptrans")
                    nc.tensor.transpose(out=pt[:],
                                        in_=erow[:, kb * 128:(kb + 1) * 128],
                                        identity=ident_f[:])
                    nc.vector.tensor_copy(h_res[:, kb, tb * 128:(tb + 1) * 128],
                                          pt[:])

            # ---------- transposed layernorm ----------
            def t_layernorm(gvec, bvec, accum_hs):
                gb_sb = biasp.tile([128, 2 * KB], F32, tag="lngb", name="lngb")
                nc.gpsimd.dma_start(out=gb_sb[:, 0:KB],
                                    in_=gvec.rearrange("(m p) -> p m", p=128))
                nc.gpsimd.dma_start(out=gb_sb[:, KB:2 * KB],
                                    in_=bvec.rearrange("(m p) -> p m", p=128))
                st0 = psum.tile([1, R], F32, tag="pctx", name="pstat0")
                st1 = psum.tile([1, R], F32, tag="pctx", name="pstat1")
                for kb in range(KB):
                    pre = scr.tile([128, R], BF16, tag="lnpre", name="lnpre",
                                   bufs=2)
                    nc.vector.tensor_copy(pre[:], h_res[:, kb, :])
                    nc.tensor.matmul(out=st0[:], lhsT=ones_b[:], rhs=pre[:],
                                     start=(kb == 0), stop=(kb == KB - 1))
                    sq = scr.tile([128, R], BF16, tag="lnsq", name="lnsq",
                                  bufs=2)
                    nc.scalar.activation(out=sq[:], in_=h_res[:, kb, :],
                                         func=AF.Square)
                    nc.tensor.matmul(out=st1[:], lhsT=ones_b[:], rhs=sq[:],
                                     start=(kb == 0), stop=(kb == KB - 1))
                mrv = scr.tile([1, 4 * R], F32, tag="lnmrv", name="lnmrv", bufs=1)
                mean = mrv[:, 0:R]
                var = mrv[:, R:2 * R]
                rstd = mrv[:, 2 * R:3 * R]
                m2 = mrv[:, 3 * R:4 * R]
                nc.vector.tensor_scalar_mul(mean, st0[:], 1.0 / D)
                nc.vector.tensor_scalar_mul(var, st1[:], 1.0 / D)
                nc.vector.tensor_mul(m2, mean, mean)
                nc.vector.scalar_tensor_tensor(out=var, in0=var, scalar=1e-5,
                                               in1=m2, op0=OP.add,
                                               op1=OP.subtract)
                nc.vector.reciprocal(out=rstd, in_=var)
                rowb = scr.tile([1, 2 * R], BF16, tag="lnrowb", name="lnrowb",
                                bufs=2)
                rstd_b = rowb[:, 0:R]
                mr_b = rowb[:, R:2 * R]
                with nc.allow_low_precision(reason="bf16 LN broadcast rows"):
                    nc.scalar.activation(out=rstd_b, in_=rstd, func=AF.Sqrt)
                    nc.vector.tensor_mul(mr_b, mean, rstd_b)
                pbr = psum.tile([128, 2 * R], F32, tag="pbig", name="pbr")
                nc.tensor.matmul(out=pbr[:, 0:R], lhsT=ones_rb[:],
                                 rhs=rstd_b, start=True, stop=True)
                nc.tensor.matmul(out=pbr[:, R:2 * R], lhsT=ones_rb[:],
                                 rhs=mr_b, start=True, stop=True)
                for kb in range(KB):
                    tmp = scr.tile([128, R], F32, tag="lntmp", name="lntmp", bufs=2)
                    nc.vector.tensor_mul(tmp[:], h_res[:, kb, :], pbr[:, 0:R])
                    nc.vector.tensor_sub(tmp[:], tmp[:], pbr[:, R:2 * R])
                    gk = gb_sb[:, kb:kb + 1]
                    bk_ = gb_sb[:, KB + kb:KB + kb + 1].to_broadcast([128, R])
                    nc.vector.scalar_tensor_tensor(out=xbf[:, kb, :], in0=tmp[:],
                                                   scalar=gk, in1=bk_,
                                                   op0=OP.mult, op1=OP.add)
                    nc.vector.scalar_tensor_tensor(out=h_res[:, kb, :],
                                                   in0=tmp[:], scalar=gk,
                                                   in1=bk_, op0=OP.mult,
                                                   op1=OP.add)
                    if accum_hs:
                        nc.vector.tensor_add(hs_sum[:, kb, :], hs_sum[:, kb, :],
                                             h_res[:, kb, :])

            t_layernorm(lne_g, lne_b, False)

            # ---------- layers ----------
            for l in range(NL):
                for kb in range(KB):
                    nc.gpsimd.dma_start(out=cc_x_in[0, kb],
                                        in_=xbf[:, kb, 0:WIN])
                    nc.gpsimd.dma_start(out=cc_x_in[1, kb],
                                        in_=xbf[:, kb, R - WIN:R])
                nc.gpsimd.collective_compute(
                    "AllGather", OP.bypass, replica_groups=groups,
                    ins=[cc_x_in[:, :, :, :]], outs=[cc_x_out[:, :, :, :]])
                for side in (0, 1):
                    for kb in range(KB):
                        nc.gpsimd.indirect_dma_start(
                            out=xh[side][:, kb, :], out_offset=None,
                            in_=ccx_rows,
                            in_offset=bass.IndirectOffsetOnAxis(
                                ap=hidx_sb[:, side, kb:kb + 1], axis=0))
                        nc.vector.tensor_scalar_mul(xh[side][:, kb, :],
                                                    xh[side][:, kb, :],
                                                    hok_sb[:, side:side + 1])

                bias_sb = biasp.tile([128, 3 * KB], F32, tag="bqko", name="bqko")
                for i, bt in enumerate((bq, bk, bo)):
                    nc.gpsimd.dma_start(out=bias_sb[:, i * KB:(i + 1) * KB],
                                        in_=bt[l].rearrange("(m p) -> p m",
                                                            p=128))

                def wpanels(wt_dram):
                    out = []
                    for kb in range(KB):
                        wt_ = wpan.tile([128, D], BF16, tag="wpan", name="wpan")
                        nc.gpsimd.dma_start(
                            out=wt_[:], in_=wt_dram[l, kb * 128:(kb + 1) * 128, :])
                        out.append(wt_)
                    return out

                # q projection (own tokens)
                wp = wpanels(wq)
                for mb in range(KB):
                    ps = psum.tile([128, R], F32, tag="pproj", name="pproj")
                    for kb in range(KB):
                        nc.tensor.matmul(out=ps[:],
                                         lhsT=wp[kb][:, mb * 128:(mb + 1) * 128],
                                         rhs=xbf[:, kb, :],
                                         start=(kb == 0), stop=(kb == KB - 1))
                    nc.scalar.activation(out=qt[:, mb, :], in_=ps[:],
                                         func=AF.Identity,
                                         bias=bias_sb[:, mb:mb + 1])
                # k projection (own + halos)
                wp = wpanels(wk)
                for mb in range(KB):
                    ps = psum.tile([128, R], F32, tag="pproj", name="pproj")
                    for kb in range(KB):
                        nc.tensor.matmul(out=ps[:],
                                         lhsT=wp[kb][:, mb * 128:(mb + 1) * 128],
                                         rhs=xbf[:, kb, :],
                                         start=(kb == 0), stop=(kb == KB - 1))
                    nc.scalar.activation(out=kt_ext[:, mb, WIN:WIN + R],
                                         in_=ps[:], func=AF.Identity,
                                         bias=bias_sb[:, KB + mb:KB + mb + 1])
                    for side in (0, 1):
                        ph = psum.tile([128, WIN], F32, tag="pproj", name="phalo")
                        for kb in range(KB):
                            nc.tensor.matmul(
                                out=ph[:],
                                lhsT=wp[kb][:, mb * 128:(mb + 1) * 128],
                                rhs=xh[side][:, kb, :],
                                start=(kb == 0), stop=(kb == KB - 1))
                        off = 0 if side == 0 else WIN + R
                        nc.scalar.activation(out=kt_ext[:, mb, off:off + WIN],
                                             in_=ph[:], func=AF.Identity,
                                             bias=bias_sb[:, KB + mb:KB + mb + 1])
                # v projection, row-major, heads interleaved with ones column
                wp = wpanels(wv)
                HTB = WIN // 128
                for etb in range(TB + 2 * HTB):
                    ps = psum.tile([128, D], F32, tag="pbig", name="pbig")
                    if etb < HTB:
                        xs = xh[0][:, :, etb * 128:(etb + 1) * 128]
                    elif etb < HTB + TB:
                        tb = etb - HTB
                        xs = xbf[:, :, tb * 128:(tb + 1) * 128]
                    else:
                        hb_ = etb - HTB - TB
                        xs = xh[1][:, :, hb_ * 128:(hb_ + 1) * 128]
                    for kb in range(KB):
                        for n0, n1 in ((0, 512), (512, D)):
                            nc.tensor.matmul(out=ps[:, n0:n1],
                                             lhsT=xs[:, kb, :],
                                             rhs=wp[kb][:, n0:n1],
                                             start=(kb == 0),
                                             stop=(kb == KB - 1))
                    vstep = v_ext[:].ap[1][0]
                    nc.vector.tensor_copy(
                        _ap(v_ext[:], vstep * etb, [[65, 12], [1, 64]]),
                        ps[:].rearrange("p (h e) -> p h e", h=12))
                # attention
                for h in range(NH):
                    mb, po = h // 2, (h % 2) * 64
                    for qc in range(QC):
                        pscore = psum.tile([128, 640], F32, tag="pbig", name="pbig")
                        for rb in range(5):
                            nc.tensor.matmul(
                                out=pscore[:, rb * 128:(rb + 1) * 128],
                                lhsT=kt_ext[po:po + 64, mb,
                                            (qc + rb) * 128:(qc + rb + 1) * 128],
                                rhs=qt[po:po + 64, mb, qc * 128:(qc + 1) * 128],
                                start=True, stop=True)
                        for rb, mi in MASK_PLAN[qc]:
                            nc.vector.tensor_add(
                                pscore[:, rb * 128:(rb + 1) * 128],
                                pscore[:, rb * 128:(rb + 1) * 128],
                                mask_sb[:, mi, :])
                        ew = scr.tile([128, 640], BF16, tag="ew", name="ew")
                        nc.scalar.activation(out=ew[:], in_=pscore[:],
                                             func=AF.Exp, scale=0.125)
                        pctx = psum.tile([128, 128], F32, tag="pctx", name="pctx")
                        vstep = v_ext[:].ap[1][0]
                        for rb in range(5):
                            vb = _ap(v_ext[:], vstep * (qc + rb) + 65 * h,
                                     [[1, 65]])
                            nc.tensor.matmul(out=pctx[0:65, :], lhsT=vb,
                                             rhs=ew[:, rb * 128:(rb + 1) * 128],
                                             start=(rb == 0), stop=(rb == 4))
                        rs = scr.tile([1, 128], BF16, tag="rsum", name="rsum")
                        with nc.allow_low_precision(reason="bf16 softmax recip"):
                            nc.vector.reciprocal(out=rs[:], in_=pctx[64:65, :])
                        prb = psum.tile([64, 128], F32, tag="pproj", name="prb")
                        nc.tensor.matmul(out=prb[:], lhsT=ones_rb[0:1, 0:64],
                                         rhs=rs[:], start=True, stop=True)
                        csb = scr.tile([64, 128], F32, tag="csb", name="csb")
                        nc.scalar.copy(out=csb[:], in_=pctx[0:64, :])
                        nc.vector.tensor_mul(
                            ctxT[po:po + 64, mb, qc * 128:(qc + 1) * 128],
                            csb[:], prb[:])
                # output projection + residual
                wp = wpanels(wo)
                for mb in range(KB):
                    ps = psum.tile([128, R], F32, tag="pproj", name="pproj")
                    for kb in range(KB):
                        nc.tensor.matmul(out=ps[:],
                                         lhsT=wp[kb][:, mb * 128:(mb + 1) * 128],
                                         rhs=ctxT[:, kb, :],
                                         start=(kb == 0), stop=(kb == KB - 1))
                    nc.vector.scalar_tensor_tensor(
                        out=h_res[:, mb, :], in0=ps[:],
                        scalar=bias_sb[:, 2 * KB + mb:2 * KB + mb + 1],
                        in1=h_res[:, mb, :], op0=OP.add, op1=OP.add)
                t_layernorm(ln1_g[l], ln1_b[l], False)
                # ffn
                b1_sb = biasp.tile([128, FB], F32, tag="b1sb", name="b1sb")
                nc.gpsimd.dma_start(out=b1_sb[:],
                                    in_=b1[l].rearrange("(m p) -> p m", p=128))
                b2_sb = biasp.tile([128, KB], F32, tag="b2sb", name="b2sb")
                nc.gpsimd.dma_start(out=b2_sb[:],
                                    in_=b2[l].rearrange("(m p) -> p m", p=128))
                w1p = []
                for kb in range(KB):
                    wt_ = w1pan.tile([128, FF], BF16, tag="w1pan", name="w1pan")
                    nc.gpsimd.dma_start(out=wt_[:],
                                        in_=w1[l, kb * 128:(kb + 1) * 128, :])
                    w1p.append(wt_)
                for mb in range(FB):
                    ps = psum.tile([128, R], F32, tag="pproj", name="pproj")
                    for kb in range(KB):
                        nc.tensor.matmul(out=ps[:],
                                         lhsT=w1p[kb][:, mb * 128:(mb + 1) * 128],
                                         rhs=xbf[:, kb, :],
                                         start=(kb == 0), stop=(kb == KB - 1))
                    # tanh-approx gelu: 2x*0.5*(1+tanh(.79788456*(y+0.044715 y^3)))
                    # (the 0.5 is folded into w2 on host)
                    gy = scr.tile([128, R], F32, tag="gy", name="gy", bufs=2)
                    gsq = scr.tile([128, R], F32, tag="gsq", name="gsq", bufs=2)
                    nc.scalar.activation(out=gy[:], in_=ps[:], func=AF.Identity,
                                         bias=b1_sb[:, mb:mb + 1])
                    nc.scalar.activation(out=gsq[:], in_=ps[:], func=AF.Square,
                                         bias=b1_sb[:, mb:mb + 1])
                    nc.vector.tensor_scalar(out=gsq[:], in0=gsq[:],
                                            scalar1=0.044715, scalar2=1.0,
                                            op0=OP.mult, op1=OP.add)
                    nc.vector.tensor_mul(gsq[:], gsq[:], gy[:])
                    nc.scalar.activation(out=gsq[:], in_=gsq[:], func=AF.Tanh,
                                         scale=0.7978845608028654)
                    nc.vector.scalar_tensor_tensor(out=gelu_sb[:, mb, :],
                                                   in0=gsq[:], scalar=1.0,
                                                   in1=gy[:], op0=OP.add,
                                                   op1=OP.mult)
                for mb in range(KB):
                    ps = psum.tile([128, R], F32, tag="pproj", name="pproj")
                    for kb in range(FB):
                        wt_ = wpan.tile([128, D], BF16, tag="wpan", name="wpan")
                        nc.gpsimd.dma_start(
                            out=wt_[:], in_=w2[l, kb * 128:(kb + 1) * 128, :])
                        nc.tensor.matmul(out=ps[:],
                                         lhsT=wt_[:, mb * 128:(mb + 1) * 128],
                                         rhs=gelu_sb[:, kb, :],
                                         start=(kb == 0), stop=(kb == FB - 1))
                    nc.vector.scalar_tensor_tensor(
                        out=h_res[:, mb, :], in0=ps[:],
                        scalar=b2_sb[:, mb:mb + 1],
                        in1=h_res[:, mb, :], op0=OP.add, op1=OP.add)
                t_layernorm(ln2_g[l], ln2_b[l], l >= NL - 4)

        # ============================ LSTM ============================
        with tc.tile_pool(name="lstm", bufs=1) as lsp, \
             tc.tile_pool(name="lscr", bufs=3) as lscr, \
             tc.tile_pool(name="lwpan", bufs=7) as lwpan:

            hs_bf = lsp.tile([128, KB, EXT], BF16, tag="hs_bf", name="hs_bf")
            for kb in range(KB):
                nc.vector.tensor_copy(hs_bf[:, kb, WARM:WARM + R],
                                      hs_sum[:, kb, :])
                nc.gpsimd.dma_start(out=cc_h_in[0, kb],
                                    in_=hs_bf[:, kb, WARM:2 * WARM])
                nc.gpsimd.dma_start(out=cc_h_in[1, kb],
                                    in_=hs_bf[:, kb, R:WARM + R])
            nc.gpsimd.collective_compute(
                "AllGather", OP.bypass, replica_groups=groups,
                ins=[cc_h_in[:, :, :, :]], outs=[cc_h_out[:, :, :, :]])
            for side in (0, 1):
                off = 0 if side == 0 else WARM + R
                for kb in range(KB):
                    nc.gpsimd.indirect_dma_start(
                        out=hs_bf[:, kb, off:off + WARM], out_offset=None,
                        in_=cch_rows,
                        in_offset=bass.IndirectOffsetOnAxis(
                            ap=hidx_sb[:, side, kb:kb + 1], axis=0))
                    nc.vector.tensor_scalar_mul(hs_bf[:, kb, off:off + WARM],
                                                hs_bf[:, kb, off:off + WARM],
                                                hok_sb[:, side:side + 1])

            fixs_sb = lsp.tile([128, 2], F32, tag="fixs_sb", name="fixs_sb")
            nc.gpsimd.dma_start(out=fixs_sb[:],
                                in_=fixs_d.rearrange("s p one -> p (s one)"))
            fixa_sb = lsp.tile([128, 2, GB], F32, tag="fixa_sb", name="fixa_sb")
            nc.gpsimd.dma_start(out=fixa_sb[:],
                                in_=fixa_d.rearrange("s p g -> p s g"))

            xpp_cm = tc.tile_pool(name="xpp", bufs=2, space="PSUM")
            lpsum = xpp_cm.__enter__()
            xp, xpe, whh_sb, hh, cst = {}, {}, {}, {}, {}
            for d_ in (0, 1):
                xp[d_] = lsp.tile([128, GB, EXT], BF16, tag=f"xp{d_}", name=f"xp{d_}")
                whh_sb[d_] = lsp.tile([128, HB, G4], BF16, tag=f"whh{d_}", name=f"whh{d_}")
                for kb in range(HB):
                    nc.gpsimd.dma_start(out=whh_sb[d_][:, kb, :],
                                        in_=whh[d_][kb * 128:(kb + 1) * 128, :])
                hh[d_] = lsp.tile([128, HHC, HB, C], BF16, tag=f"hh{d_}", name=f"hh{d_}")
                cst[d_] = lsp.tile([128, HB * C], F32, tag=f"c{d_}", name=f"c{d_}")
                nc.vector.memset(cst[d_][:], 0.0)

                bl_sb = lsp.tile([128, GB], F32, tag=f"bl{d_}", name=f"bl{d_}")
                nc.gpsimd.dma_start(out=bl_sb[:],
                                    in_=blstm[d_].rearrange("(m p) -> p m",
                                                            p=128))
                wps = []
                for kb in range(KB):
                    wt_ = lwpan.tile([128, G4], BF16, tag="wihpan", name="wihpan")
                    nc.gpsimd.dma_start(out=wt_[:],
                                        in_=wih[d_][kb * 128:(kb + 1) * 128, :])
                    wps.append(wt_)
                for mb in range(GB):
                    psx = lpsum.tile([128, EXT], F32, tag="pxp", name="pxp")
                    for kb in range(KB):
                        for n0, n1 in ((0, 512), (512, EXT)):
                            nc.tensor.matmul(
                                out=psx[:, n0:n1],
                                lhsT=wps[kb][:, mb * 128:(mb + 1) * 128],
                                rhs=hs_bf[:, kb, n0:n1],
                                start=(kb == 0), stop=(kb == KB - 1))
                    nc.scalar.activation(out=xp[d_][:, mb, :], in_=psx[:],
                                         func=AF.Identity,
                                         bias=bl_sb[:, mb:mb + 1])
                fcol = 0 if d_ == 0 else EXT - WARM
                fa = _ap(fixa_sb[:], GB * d_, [[1, GB], [0, WARM]])
                nc.vector.scalar_tensor_tensor(
                    out=xp[d_][:, :, fcol:fcol + WARM],
                    in0=xp[d_][:, :, fcol:fcol + WARM],
                    scalar=fixs_sb[:, d_:d_ + 1], in1=fa,
                    op0=OP.mult, op1=OP.add)
                # expand to tick-major layout (matmul rhs needs one free dim):
                # fwd: xpe[:,u,m,j] = xp[:,m,T*j+u]; bwd: ... = xp[:,m,T*j+WARM+u]
                xpe[d_] = lsp.tile([128, TICKS, GB, C], BF16,
                                   tag=f"xpe{d_}", name=f"xpe{d_}")
                for j in range(C):
                    off = T * j if d_ == 0 else T * j + WARM
                    nc.vector.tensor_copy(
                        xpe[d_][:, :, :, j],
                        _ap(xp[d_], off, [[1, TICKS], [EXT, GB]]))
                nc.vector.memset(hh[d_][:, 0:1, :, :], 0.0)
                nc.vector.memset(hh[d_][:, HHC - 1:HHC, :, :], 0.0)

            xpp_cm.__exit__(None, None, None)
            gpsum_cm = tc.tile_pool(name="gpsum", bufs=2, space="PSUM")
            gpsum = gpsum_cm.__enter__()

            # ---- recurrence ----
            # fwd: tick t reads xp ext col (T*j + t), h col t, writes h col t+1
            # bwd: tick t reads xp ext col (T*j + T+2*WARM-1 - t), h col
            #      TICKS+1-t, writes h col TICKS-t.  (cols 0 / HHC-1 are zeros)
            for t in range(TICKS):
                for d_ in (0, 1):
                    if d_ == 0:
                        xoff, wcol = t, t + 1
                        rcol = wcol - 1
                    else:
                        xoff, wcol = (T + 2 * WARM - 1) - t, TICKS - t
                        rcol = wcol + 1
                    gates = gpsum.tile([128, GB * C], F32, tag=f"gates{d_}", name=f"gates{d_}")
                    u = t if d_ == 0 else TICKS - 1 - t
                    nc.tensor.matmul(out=gates[:], lhsT=ident_b[:],
                                     rhs=xpe[d_][:, u, :, :],
                                     start=True, stop=False,
                                     skip_group_check=True)
                    for kb in range(HB):
                        rhs_h = hh[d_][:, rcol, kb, :]
                        for mb in range(GB):
                            nc.tensor.matmul(
                                out=gates[:, mb * C:(mb + 1) * C],
                                lhsT=whh_sb[d_][:, kb, mb * 128:(mb + 1) * 128],
                                rhs=rhs_h, start=False, stop=(kb == HB - 1),
                                skip_group_check=True)
                    sg = lscr.tile([128, 9 * C], F32, tag=f"sg{d_}", name=f"sg{d_}")
                    tg = lscr.tile([128, 3 * C], F32, tag=f"tg{d_}", name=f"tg{d_}")
                    nc.scalar.activation(out=sg[:], in_=gates[:, 0:9 * C],
                                         func=AF.Sigmoid)
                    nc.scalar.activation(out=tg[:], in_=gates[:, 9 * C:12 * C],
                                         func=AF.Tanh)
                    u = lscr.tile([128, 3 * C], F32, tag=f"u{d_}", name=f"u{d_}")
                    cs = cst[d_][:]
                    nc.vector.tensor_mul(u[:], sg[:, 0:3 * C], tg[:])
                    nc.vector.tensor_mul(cs, sg[:, 3 * C:6 * C], cs)
                    nc.vector.tensor_add(cs, cs, u[:])
                    tcn = lscr.tile([128, 3 * C], F32, tag=f"tc{d_}", name=f"tc{d_}")
                    nc.scalar.activation(out=tcn[:], in_=cs, func=AF.Tanh)
                    nc.vector.tensor_mul(hh[d_][:, wcol, :, :],
                                         sg[:, 6 * C:9 * C], tcn[:])

            # ---- tok assembly (row-major per token block) + event sums ----
            tok = lsp.tile([128, TB, 2 * HB, 128], BF16, tag="tok", name="tok")
            nch = 128 // T
            for tb in range(TB):
                for d_ in (0, 1):
                    c0 = WARM + 1 if d_ == 0 else 1
                    for kb in range(HB):
                        pt = gpsum.tile([128, 128], BF16, tag="ptok", name="ptok")
                        for jj in range(nch):
                            src = _ap(hh[d_],
                                      c0 * HB * C + kb * C + tb * nch + jj,
                                      [[HB * C, T]])
                            nc.tensor.transpose(out=pt[jj * T:(jj + 1) * T, :],
                                                in_=src, identity=ident_b[:])
                        nc.vector.tensor_copy(tok[:, tb, d_ * HB + kb, :],
                                              pt[:])
            mt_sb = lsp.tile([128, TB, NE], BF16, tag="mt_sb", name="mt_sb")
            nc.gpsimd.dma_start(out=mt_sb[:],
                                in_=mt_d.rearrange("(b p) e -> p b e", p=128))
            gpsum_cm.__exit__(None, None, None)
            hdp_cm = tc.tile_pool(name="hdp", bufs=1, space="PSUM")
            lpsum = hdp_cm.__enter__()
            pev = lpsum.tile([NE, D], F32, tag="pev", name="pev")
            tstep = tok[:].ap[1][0]
            for tb in range(TB):
                for n0, n1 in ((0, 512), (512, D)):
                    nc.tensor.matmul(
                        out=pev[:, n0:n1], lhsT=mt_sb[:, tb, :],
                        rhs=_ap(tok[:], tstep * tb + n0, [[1, n1 - n0]]),
                        start=(tb == 0), stop=(tb == TB - 1))
            ev_part = lsp.tile([NE, D], F32, tag="ev_part", name="ev_part")
            nc.vector.tensor_copy(ev_part[:], pev[:])
            nc.gpsimd.dma_start(out=cc_ev_in[:, :], in_=ev_part[:])
            nc.gpsimd.collective_compute(
                "AllReduce", OP.add, replica_groups=groups,
                ins=[cc_ev_in[:, :]], outs=[cc_ev_out[:, :]])
            ev_f = lsp.tile([NE, D], F32, tag="ev_f", name="ev_f")
            ev_bf = lsp.tile([NE, D], BF16, tag="ev_bf", name="ev_bf")
            il_sb = lsp.tile([NE, 1], F32, tag="il_sb", name="il_sb")
            nc.gpsimd.dma_start(out=il_sb[:], in_=inv_len[:, :])
            nc.gpsimd.dma_start(out=ev_f[:], in_=cc_ev_out[:, :])
            nc.vector.tensor_scalar_mul(ev_bf[:], ev_f[:], il_sb[:])

            # ---- pair MLP ----
            feats = lsp.tile([128, 4 * KB, PPC], BF16, tag="feats", name="feats")
            g1_sb = lsp.tile([NE, PPC], BF16, tag="g1_sb", name="g1_sb")
            g2_sb = lsp.tile([NE, PPC], BF16, tag="g2_sb", name="g2_sb")
            nc.gpsimd.dma_start(out=g1_sb[:], in_=g1t_d[:, :])
            nc.gpsimd.dma_start(out=g2_sb[:], in_=g2t_d[:, :])
            for mb in range(KB):
                p1 = lpsum.tile([128, PPC], F32, tag="pe1", name="pe1")
                p2 = lpsum.tile([128, PPC], F32, tag="pe2", name="pe2")
                nc.tensor.matmul(out=p1[:], lhsT=ev_bf[:, mb * 128:(mb + 1) * 128],
                                 rhs=g1_sb[:], start=True, stop=True)
                nc.tensor.matmul(out=p2[:], lhsT=ev_bf[:, mb * 128:(mb + 1) * 128],
                                 rhs=g2_sb[:], start=True, stop=True)
                nc.vector.tensor_copy(feats[:, mb, :], p1[:])
                nc.vector.tensor_copy(feats[:, KB + mb, :], p2[:])
                nc.vector.tensor_sub(feats[:, 2 * KB + mb, :],
                                     feats[:, mb, :], feats[:, KB + mb, :])
                nc.vector.tensor_mul(feats[:, 3 * KB + mb, :],
                                     feats[:, mb, :], p2[:])
            hb1 = lsp.tile([128, KB], F32, tag="hb1", name="hb1")
            nc.gpsimd.dma_start(out=hb1[:],
                                in_=h1b.rearrange("(m p) -> p m", p=128))
            r1 = lsp.tile([128, KB, PPC], BF16, tag="r1", name="r1")
            for mb in range(KB):
                ps = lpsum.tile([128, PPC], F32, tag="ph1", name="ph1")
                for kb in range(4 * KB):
                    wt_ = lwpan.tile([128, D], BF16, tag="h1pan", name="h1pan")
                    nc.gpsimd.dma_start(out=wt_[:],
                                        in_=h1w[kb * 128:(kb + 1) * 128, :])
                    nc.tensor.matmul(out=ps[:],
                                     lhsT=wt_[:, mb * 128:(mb + 1) * 128],
                                     rhs=feats[:, kb, :],
                                     start=(kb == 0), stop=(kb == 4 * KB - 1))
                nc.scalar.activation(out=r1[:, mb, :], in_=ps[:], func=AF.Relu,
                                     bias=hb1[:, mb:mb + 1])
            hb2 = lsp.tile([128, 2], F32, tag="hb2", name="hb2")
            nc.gpsimd.dma_start(out=hb2[:],
                                in_=h2b.rearrange("(m p) -> p m", p=128))
            w2p_sb = lsp.tile([128, KB, 256], BF16, tag="h2pan", name="h2pan")
            for kb in range(KB):
                nc.gpsimd.dma_start(out=w2p_sb[:, kb, :],
                                    in_=h2w[kb * 128:(kb + 1) * 128, :])
            r2 = lsp.tile([128, 2, PPC], BF16, tag="r2", name="r2")
            for mb in range(2):
                ps = lpsum.tile([128, PPC], F32, tag="ph2", name="ph2")
                for kb in range(KB):
                    nc.tensor.matmul(out=ps[:],
                                     lhsT=w2p_sb[:, kb, mb * 128:(mb + 1) * 128],
                                     rhs=r1[:, kb, :],
                                     start=(kb == 0), stop=(kb == KB - 1))
                nc.scalar.activation(out=r2[:, mb, :], in_=ps[:], func=AF.Relu,
                                     bias=hb2[:, mb:mb + 1])
            w3_sb = lsp.tile([128, 2, 2], BF16, tag="h3sb", name="h3sb")
            for kb in range(2):
                nc.gpsimd.dma_start(out=w3_sb[:, kb, :],
                                    in_=h3w[kb * 128:(kb + 1) * 128, :])
            pl = lpsum.tile([2, PPC], F32, tag="plog", name="plog")
            for kb in range(2):
                nc.tensor.matmul(out=pl[:], lhsT=w3_sb[:, kb, :],
                                 rhs=r2[:, kb, :],
                                 start=(kb == 0), stop=(kb == 1))
            b3_sb = lsp.tile([2, 1], F32, tag="b3sb", name="b3sb")
            nc.gpsimd.dma_start(out=b3_sb[:],
                                in_=h3b.rearrange("(a b) -> a b", b=1))
            logit = lsp.tile([2, PPC], F32, tag="logit", name="logit")
            nc.vector.tensor_scalar_add(logit[:], pl[:], b3_sb[:])
            nc.sync.dma_start(out=logits_o[:, :], in_=logit[:])
            # loss = sum_j w_j * (softplus(d_j) - d_j*y_j),  d = l1 - l0
            y_sb = lsp.tile([1, PPC], F32, tag="y_sb", name="y_sb")
            w_sb = lsp.tile([1, PPC], F32, tag="w_sb", name="w_sb")
            nc.gpsimd.dma_start(out=y_sb[:], in_=yw_d[0:1, :])
            nc.gpsimd.dma_start(out=w_sb[:], in_=yw_d[1:2, :])
            pm1_sb = lsp.tile([2, 1], F32, tag="pm1_sb", name="pm1_sb")
            nc.gpsimd.dma_start(out=pm1_sb[:], in_=pm1_d[:, :])
            pd = lpsum.tile([1, PPC], F32, tag="pdif", name="pdif")
            nc.tensor.matmul(out=pd[:], lhsT=pm1_sb[:], rhs=logit[:],
                             start=True, stop=True)
            dvec = lsp.tile([1, 3 * PPC], F32, tag="dvec", name="dvec")
            dd = dvec[:, 0:PPC]
            sp = dvec[:, PPC:2 * PPC]
            nl_ = dvec[:, 2 * PPC:3 * PPC]
            nc.vector.tensor_copy(dd, pd[:])
            nc.scalar.activation(out=sp, in_=dd, func=AF.Exp)
            nc.vector.tensor_scalar_add(sp, sp, 1.0)
            nc.scalar.activation(out=sp, in_=sp, func=AF.Ln)
            nc.vector.tensor_mul(nl_, dd, y_sb[:])
            nc.vector.tensor_sub(nl_, sp, nl_)
            lacc = lsp.tile([1, 1], F32, tag="lacc", name="lacc")
            nc.vector.scalar_tensor_tensor(out=nl_, in0=nl_, scalar=0.0,
                                           in1=w_sb[:], op0=OP.bypass,
                                           op1=OP.mult, accum_out=lacc[:])
            nc.sync.dma_start(out=loss_o[:, :], in_=lacc[:])
            hdp_cm.__exit__(None, None, None)

    nc.compile()
    return nc


_BUILD_CACHE = {}


def _get_nc(cfg: Cfg):
    if cfg not in _BUILD_CACHE:
        _BUILD_CACHE[cfg] = build(cfg)
    return _BUILD_CACHE[cfg]


def assemble(cfg: Cfg, results):
    loss = np.float32(0.0)
    logits = np.zeros((cfg.NPAIR, 2), np.float32)
    for c in range(cfg.NC):
        out = results[c]
        loss += np.float32(out["loss_part"].reshape(()))
        lo = c * cfg.RPC
        hi = min(lo + cfg.RPC, cfg.NPAIR)
        logits[lo:hi] = np.asarray(out["logitsT"], np.float32).T[: hi - lo]
    return np.float32(loss), logits


def run(inputs, cfg: Cfg = Cfg(), trace=False):
    in_maps = _prep(inputs, cfg)
    nc = _get_nc(cfg)
    kw = {}
    if trace:
        kw = dict(trace=True, trace_cores=list(range(cfg.NC)))
    res = bass_utils.run_bass_kernel_spmd(nc, in_maps,
                                          core_ids=list(range(cfg.NC)), **kw)
    return assemble(cfg, res.results), res


def kernel(**inputs):
    (loss, logits), _ = run(inputs, Cfg())
    return loss, logits
